# revision 10
# baseline (speedup 1.0000x reference)
"""Distributed memory-shard scale kernel for Trainium2 (8 NeuronCores).

Computes out[b, s, d] = x[b, s, d] * shards[shard_map[d], d] for
x: [4, 4096, 4096] f32, shards: [8, 4096] f32, shard_map: [4096] int.

Strategy: data-parallel over the flattened (batch*seq) rows — each of the
8 cores owns a contiguous 2048-row slice of x and replicates the tiny
shards/shard_map inputs. The kernel is DMA-bandwidth-bound (pure
elementwise scale), so the x stream is staged in bf16 (host casts, device
streams/multiplies/stores bf16, host upcasts): 64MB -> 32MB of DMA per
core for a ~2.9e-3 relative error, inside the 2e-2 budget.

On device each core:
  1. loads aux[s, :] = [shard_map - s | shards[s]] (bf16, 8 partitions)
     via the otherwise-idle GPSIMD SWDGE ring,
  2. builds masked products B[s, d] = (shard_map[d]==s) * shards[s, d]
     with ONE fused scalar_tensor_tensor, then reduces over shards AND
     broadcasts to all 128 partitions in one step: matmul
     ones[8,128].T @ B[8,512] -> PSUM[128,512] per chunk. PSUM->SBUF bf16
     casts alternate DVE/ACT so early w chunks unblock the first muls,
  3. streams x through SBUF in [128, 2*4096] bf16 tiles (two consecutive
     rows per partition, 16KB contiguous lines), multiplying by w on DVE
     and storing on the ACT HWDGE ring.

SDMA engine 15 (serving SBUF partitions 92-95/124-127 per the port
swizzle port=((p>>2)&7)<<1|(p>>6)) is ~20% slower per byte than the
other 15 engines and paces the whole kernel if every partition carries
equal bytes. The last 256-row block therefore avoids those partitions:
it is carved into slices on partitions 0-91, 96-123, and 16-31, cutting
engine 15's share from 16 to 14 rows per partition, which balances its
(slower) drain against the other engines. These slices are interleaved
between the big tiles to hide their DVE cost in idle gaps.
"""

import numpy as np
import ml_dtypes

import bass_rust as _bass_rust
import concourse.bass as bass
import concourse.tile as tile
from concourse import mybir
from concourse.bass_utils import run_bass_kernel_spmd

N_CORES = 8
BATCH, SEQ, DIM = 4, 4096, 4096
NUM_SHARDS = 8
ROWS_TOTAL = BATCH * SEQ               # 16384
ROWS_PER_CORE = ROWS_TOTAL // N_CORES  # 2048
P = 128                                # SBUF partitions

BF16 = ml_dtypes.bfloat16

TRACE = False       # set True (e.g. from test.py) to capture an NTFF profile
LAST_RESULT = None  # BassKernelResults of the most recent kernel() call

_cached_nc = None


def _build_program() -> bass.Bass:
    f32 = mybir.dt.float32
    bf16 = mybir.dt.bfloat16
    nc = bass.Bass()
    x_in = nc.dram_tensor("x", [ROWS_PER_CORE, DIM], bf16, kind="ExternalInput")
    # aux[s, 0:DIM]     = shard_map - s   (bf16-exact: values in [-7, 7])
    # aux[s, DIM:2*DIM] = shards[s, :]
    aux_in = nc.dram_tensor("aux", [NUM_SHARDS, 2 * DIM], bf16,
                            kind="ExternalInput")
    out = nc.dram_tensor("out", [ROWS_PER_CORE, DIM], bf16,
                         kind="ExternalOutput")

    with tile.TileContext(nc) as tc:
        with tc.tile_pool(name="const", bufs=1) as cpool, \
             tc.tile_pool(name="xp", bufs=7) as xpool:
            # aux rides the idle GPSIMD SWDGE ring: the sync HWDGE ring
            # starts streaming x immediately, stores own the ACT ring.
            auxt = cpool.tile([NUM_SHARDS, 2 * DIM], bf16)
            nc.gpsimd.dma_start(auxt[:], aux_in[:])
            ones8 = cpool.tile([NUM_SHARDS, P], bf16)
            nc.vector.memset(ones8[:], 1.0)
            # small SB->SB transfer to warm up the ACT HWDGE ring before
            # the first real store needs it (>=512B to stay on the
            # fast path)
            warm = cpool.tile([1, 512], bf16)
            nc.vector.memset(warm[:, 0:256], 0.0)
            nc.scalar.dma_start(warm[:, 256:512], warm[:, 0:256])

            # B[s, d] = (shard_map[d] - s == 0) * shards[s, d], in place
            # over the shard_map half of aux.
            nc.vector.scalar_tensor_tensor(
                out=auxt[:, 0:DIM], in0=auxt[:, 0:DIM], scalar=0.0,
                in1=auxt[:, DIM:2 * DIM],
                op0=mybir.AluOpType.is_equal, op1=mybir.AluOpType.mult)

            # w[d] = sum_s B[s, d], replicated to 128 partitions by the
            # ones[8,128] stationary: PSUM[p, d] = sum_s ones[s,p]*B[s,d].
            w128 = cpool.tile([P, DIM], bf16)
            MMF = 512  # one PSUM bank per matmul
            with tc.tile_pool(name="ps", bufs=8, space="PSUM") as ppool:
                for k in range(DIM // MMF):
                    mm = ppool.tile([P, MMF], f32)
                    nc.tensor.matmul(mm[:], ones8[:],
                                     auxt[:, k * MMF:(k + 1) * MMF],
                                     start=True, stop=True)
                    if k % 2 == 0:
                        nc.vector.tensor_copy(w128[:, k * MMF:(k + 1) * MMF],
                                              mm[:])
                    else:
                        nc.scalar.copy(w128[:, k * MMF:(k + 1) * MMF], mm[:])

            # --- stream x through SBUF, scaling by w ---
            # 7 big tiles cover rows 0..1791 on all 128 partitions. The
            # last 256 rows go to engine-15-free partition slices:
            #   7a: rows 1792..1975 on partitions 0..91   ([92, 8192])
            #   7b: rows 1976..2031 on partitions 96..123 ([28, 8192])
            #   7c: rows 2032..2047 on partitions 0..15   ([16, 4096])
            x2v = x_in.rearrange("(i p t) d -> i p (t d)", p=P, t=2)
            o2v = out.rearrange("(i p t) d -> i p (t d)", p=P, t=2)
            xr2 = x_in.rearrange("(r t) d -> r (t d)", t=2)   # [1024, 8192]
            or2 = out.rearrange("(r t) d -> r (t d)", t=2)
            N_BIG = 7
            QW = DIM // 2                      # quarter width (2048)

            t7 = None

            def small_piece(which):
                nonlocal t7
                if which == 0:      # 7a
                    if t7 is None:
                        t7 = xpool.tile([124, 2 * DIM], bf16, name="t7",
                                        bufs=1)
                    sl = t7[0:92, :]
                    nc.sync.dma_start(sl, xr2[896:988])
                    for h in range(2):
                        cols = slice(h * DIM, (h + 1) * DIM)
                        nc.vector.tensor_mul(sl[:, cols], sl[:, cols],
                                             w128[0:92, :])
                    nc.scalar.dma_start(or2[896:988], sl)
                elif which == 1:    # 7b
                    sl = t7[96:124, :]
                    nc.sync.dma_start(sl, xr2[988:1016])
                    for h in range(2):
                        cols = slice(h * DIM, (h + 1) * DIM)
                        nc.vector.tensor_mul(sl[:, cols], sl[:, cols],
                                             w128[96:124, :])
                    nc.scalar.dma_start(or2[988:1016], sl)
                else:               # 7c
                    t7c = xpool.tile([16, DIM], bf16, name="t7c", bufs=1)
                    sl = t7c[:]
                    nc.sync.dma_start(sl, x_in[2032:2048])
                    nc.vector.tensor_mul(sl, sl, w128[0:16, :])
                    nc.scalar.dma_start(out[2032:2048], sl)

            for i in range(N_BIG):
                xt = xpool.tile([P, 2 * DIM], bf16)
                if i == 0 or i == N_BIG - 1:
                    # quarter chunks: first tile so the store stream
                    # starts early, last tile to shorten the drain
                    if i == 0:
                        for q in range(4):
                            cols = slice(q * QW, (q + 1) * QW)
                            nc.sync.dma_start(xt[:, cols], x2v[i, :, cols])
                    else:
                        nc.sync.dma_start(xt[:], x2v[i])
                    for q in range(4):
                        cols = slice(q * QW, (q + 1) * QW)
                        nc.vector.tensor_mul(
                            xt[:, cols], xt[:, cols],
                            w128[:, (q % 2) * QW:(q % 2 + 1) * QW])
                        nc.scalar.dma_start(o2v[i, :, cols], xt[:, cols])
                else:
                    nc.sync.dma_start(xt[:], x2v[i])
                    for h in range(2):
                        cols = slice(h * DIM, (h + 1) * DIM)
                        nc.vector.tensor_mul(xt[:, cols], xt[:, cols],
                                             w128[:])
                    nc.scalar.dma_start(o2v[i], xt[:])
                # interleave the engine-15-free slices mid-stream so
                # their DVE cost hides in idle gaps
                if i == 3:
                    small_piece(0)
                elif i == 4:
                    small_piece(1)
                elif i == 5:
                    small_piece(2)
    # TRN2 allows one sync wait per instruction; split multi-wait
    # instructions the way bacc's compile pipeline does.
    _bass_rust.generate_event_semaphores(nc)
    return nc


def _marshal(shards: np.ndarray, shard_map: np.ndarray):
    sh = np.asarray(shards, dtype=np.float32)
    sm = np.asarray(shard_map).astype(np.float32)
    aux = np.empty((NUM_SHARDS, 2 * DIM), dtype=BF16)
    aux[:, 0:DIM] = (sm[None, :]
                     - np.arange(NUM_SHARDS, dtype=np.float32)[:, None]
                     ).astype(BF16)
    aux[:, DIM:] = sh.astype(BF16)
    return aux


def kernel(x, shards, shard_map):
    global _cached_nc, LAST_RESULT
    if _cached_nc is None:
        _cached_nc = _build_program()
    nc = _cached_nc

    x2 = np.asarray(x, dtype=np.float32).reshape(ROWS_TOTAL, DIM).astype(BF16)
    aux = _marshal(shards, shard_map)

    in_maps = [
        {"x": x2[c * ROWS_PER_CORE:(c + 1) * ROWS_PER_CORE], "aux": aux}
        for c in range(N_CORES)
    ]
    res = run_bass_kernel_spmd(nc, in_maps, core_ids=list(range(N_CORES)),
                               trace=TRACE)
    LAST_RESULT = res
    out = np.concatenate([r["out"] for r in res.results], axis=0)
    return out.astype(np.float32).reshape(BATCH, SEQ, DIM)


# revision 11
# speedup vs baseline: 1.3125x; 1.3125x over previous
"""Distributed memory-shard scale kernel for Trainium2 (8 NeuronCores).

Computes out[b, s, d] = x[b, s, d] * shards[shard_map[d], d] for
x: [4, 4096, 4096] f32, shards: [8, 4096] f32, shard_map: [4096] int.

Strategy: data-parallel over the flattened (batch*seq) rows — each of the
8 cores owns a contiguous 2048-row slice of x and replicates the tiny
shards/shard_map inputs. The kernel is DMA-bandwidth-bound (pure
elementwise scale), so the x stream is staged in bf16 (host casts, device
streams/multiplies/stores bf16, host upcasts): 64MB -> 32MB of DMA per
core for a ~2.9e-3 relative error, inside the 2e-2 budget.

On device each core:
  1. loads aux[s, :] = [shard_map - s | shards[s]] (bf16, 8 partitions)
     via the otherwise-idle GPSIMD SWDGE ring,
  2. builds masked products B[s, d] = (shard_map[d]==s) * shards[s, d]
     with ONE fused scalar_tensor_tensor, then reduces over shards AND
     broadcasts to all 128 partitions in one step: matmul
     ones[8,128].T @ B[8,512] -> PSUM[128,512] per chunk. PSUM->SBUF bf16
     casts alternate DVE/ACT so early w chunks unblock the first muls,
  3. streams x through SBUF in [128, 4*4096] bf16 tiles — four
     consecutive rows per partition = 32KB contiguous per-partition
     lines. Descriptor size matters: SDMA engine 15 processes 16KB
     descriptors ~20% slower than the other engines (it paced the whole
     kernel when tiles had 16KB lines), but at 32KB and 8KB all 16
     engines run at full ~26.7 GB/s. Muls run per row-column chunk
     ([128, 4096]) on DVE; stores ride the ACT HWDGE ring. The first
     tile loads/muls/stores in [128, 4096] row chunks (8KB lines) so
     the store stream starts early; the last tile stores in row chunks
     to shorten the final drain.
"""

import numpy as np
import ml_dtypes

import bass_rust as _bass_rust
import concourse.bass as bass
import concourse.tile as tile
from concourse import mybir
from concourse.bass_utils import run_bass_kernel_spmd

N_CORES = 8
BATCH, SEQ, DIM = 4, 4096, 4096
NUM_SHARDS = 8
ROWS_TOTAL = BATCH * SEQ               # 16384
ROWS_PER_CORE = ROWS_TOTAL // N_CORES  # 2048
P = 128                                # SBUF partitions
T = 4                                  # rows per partition per tile
N_BIG = ROWS_PER_CORE // (T * P)       # 4 tiles

BF16 = ml_dtypes.bfloat16

TRACE = False       # set True (e.g. from test.py) to capture an NTFF profile
LAST_RESULT = None  # BassKernelResults of the most recent kernel() call

_cached_nc = None


def _build_program() -> bass.Bass:
    f32 = mybir.dt.float32
    bf16 = mybir.dt.bfloat16
    nc = bass.Bass()
    x_in = nc.dram_tensor("x", [ROWS_PER_CORE, DIM], bf16, kind="ExternalInput")
    # aux[s, 0:DIM]     = shard_map - s   (bf16-exact: values in [-7, 7])
    # aux[s, DIM:2*DIM] = shards[s, :]
    aux_in = nc.dram_tensor("aux", [NUM_SHARDS, 2 * DIM], bf16,
                            kind="ExternalInput")
    out = nc.dram_tensor("out", [ROWS_PER_CORE, DIM], bf16,
                         kind="ExternalOutput")

    with tile.TileContext(nc) as tc:
        with tc.tile_pool(name="const", bufs=1) as cpool, \
             tc.tile_pool(name="xp", bufs=4) as xpool:
            # aux rides the idle GPSIMD SWDGE ring: the sync HWDGE ring
            # starts streaming x immediately, stores own the ACT ring.
            auxt = cpool.tile([NUM_SHARDS, 2 * DIM], bf16)
            nc.gpsimd.dma_start(auxt[:], aux_in[:])
            ones8 = cpool.tile([NUM_SHARDS, P], bf16)
            nc.vector.memset(ones8[:], 1.0)
            # small SB->SB transfer to warm up the ACT HWDGE ring before
            # the first real store needs it
            warm = cpool.tile([1, 512], bf16)
            nc.vector.memset(warm[:, 0:256], 0.0)
            nc.scalar.dma_start(warm[:, 256:512], warm[:, 0:256])

            # B[s, d] = (shard_map[d] - s == 0) * shards[s, d], in place
            # over the shard_map half of aux.
            nc.vector.scalar_tensor_tensor(
                out=auxt[:, 0:DIM], in0=auxt[:, 0:DIM], scalar=0.0,
                in1=auxt[:, DIM:2 * DIM],
                op0=mybir.AluOpType.is_equal, op1=mybir.AluOpType.mult)

            # w[d] = sum_s B[s, d], replicated to 128 partitions by the
            # ones[8,128] stationary: PSUM[p, d] = sum_s ones[s,p]*B[s,d].
            w128 = cpool.tile([P, DIM], bf16)
            MMF = 512  # one PSUM bank per matmul
            with tc.tile_pool(name="ps", bufs=8, space="PSUM") as ppool:
                for k in range(DIM // MMF):
                    mm = ppool.tile([P, MMF], f32)
                    nc.tensor.matmul(mm[:], ones8[:],
                                     auxt[:, k * MMF:(k + 1) * MMF],
                                     start=True, stop=True)
                    if k % 2 == 0:
                        nc.vector.tensor_copy(w128[:, k * MMF:(k + 1) * MMF],
                                              mm[:])
                    else:
                        nc.scalar.copy(w128[:, k * MMF:(k + 1) * MMF], mm[:])

            # --- stream x through SBUF, scaling by w ---
            # Column chunk r of a tile is row T*p+r, so every [128, 4096]
            # row chunk multiplies against w128 directly.
            x4v = x_in.rearrange("(i p t) d -> i p (t d)", p=P, t=T)
            o4v = out.rearrange("(i p t) d -> i p (t d)", p=P, t=T)
            for i in range(N_BIG):
                xt = xpool.tile([P, T * DIM], bf16)
                first, last = i == 0, i == N_BIG - 1
                if first:
                    # row-chunk loads (8KB lines) so the pipeline ramps
                    for r in range(T):
                        cols = slice(r * DIM, (r + 1) * DIM)
                        nc.sync.dma_start(xt[:, cols], x4v[i, :, cols])
                else:
                    nc.sync.dma_start(xt[:], x4v[i])
                for r in range(T):
                    cols = slice(r * DIM, (r + 1) * DIM)
                    nc.vector.tensor_mul(xt[:, cols], xt[:, cols], w128[:])
                    if first or last:
                        # chunked stores: early store start / short drain
                        nc.scalar.dma_start(o4v[i, :, cols], xt[:, cols])
                if not (first or last):
                    nc.scalar.dma_start(o4v[i], xt[:])
    # TRN2 allows one sync wait per instruction; split multi-wait
    # instructions the way bacc's compile pipeline does.
    _bass_rust.generate_event_semaphores(nc)
    return nc


def _marshal(shards: np.ndarray, shard_map: np.ndarray):
    sh = np.asarray(shards, dtype=np.float32)
    sm = np.asarray(shard_map).astype(np.float32)
    aux = np.empty((NUM_SHARDS, 2 * DIM), dtype=BF16)
    aux[:, 0:DIM] = (sm[None, :]
                     - np.arange(NUM_SHARDS, dtype=np.float32)[:, None]
                     ).astype(BF16)
    aux[:, DIM:] = sh.astype(BF16)
    return aux


def kernel(x, shards, shard_map):
    global _cached_nc, LAST_RESULT
    if _cached_nc is None:
        _cached_nc = _build_program()
    nc = _cached_nc

    x2 = np.asarray(x, dtype=np.float32).reshape(ROWS_TOTAL, DIM).astype(BF16)
    aux = _marshal(shards, shard_map)

    in_maps = [
        {"x": x2[c * ROWS_PER_CORE:(c + 1) * ROWS_PER_CORE], "aux": aux}
        for c in range(N_CORES)
    ]
    res = run_bass_kernel_spmd(nc, in_maps, core_ids=list(range(N_CORES)),
                               trace=TRACE)
    LAST_RESULT = res
    out = np.concatenate([r["out"] for r in res.results], axis=0)
    return out.astype(np.float32).reshape(BATCH, SEQ, DIM)


# revision 12
# speedup vs baseline: 1.3485x; 1.0275x over previous
"""Distributed memory-shard scale kernel for Trainium2 (8 NeuronCores).

Computes out[b, s, d] = x[b, s, d] * shards[shard_map[d], d] for
x: [4, 4096, 4096] f32, shards: [8, 4096] f32, shard_map: [4096] int.

Strategy: data-parallel over the flattened (batch*seq) rows — each of the
8 cores owns a contiguous 2048-row slice of x and replicates the tiny
shards/shard_map inputs. The kernel is DMA-bandwidth-bound (pure
elementwise scale), so the x stream is staged in bf16 (host casts, device
streams/multiplies/stores bf16, host upcasts): 64MB -> 32MB of DMA per
core for a ~2.9e-3 relative error, inside the 2e-2 budget.

On device each core:
  1. loads aux[s, :] = [shard_map - s | shards[s]] (bf16, 8 partitions)
     via the otherwise-idle GPSIMD SWDGE ring,
  2. builds masked products B[s, d] = (shard_map[d]==s) * shards[s, d]
     with ONE fused scalar_tensor_tensor, then reduces over shards AND
     broadcasts to all 128 partitions in one step: matmul
     ones[8,128].T @ B[8,512] -> PSUM[128,512] per chunk. PSUM->SBUF bf16
     casts alternate DVE/ACT so early w chunks unblock the first muls,
  3. streams x through SBUF in [128, 4*4096] bf16 tiles — four
     consecutive rows per partition = 32KB contiguous per-partition
     lines. Descriptor size matters: SDMA engine 15 processes 16KB
     descriptors ~20% slower than the other engines (it paced the whole
     kernel when tiles had 16KB lines), but at 32KB and 8KB all 16
     engines run at full ~26.7 GB/s. Muls run per row-column chunk
     ([128, 4096]) on DVE; stores ride the ACT HWDGE ring. The first
     tile loads/muls/stores in [128, 4096] row chunks (8KB lines) so
     the store stream starts early; the last tile stores in row chunks
     to shorten the final drain.
"""

import numpy as np
import ml_dtypes

import bass_rust as _bass_rust
import concourse.bass as bass
import concourse.tile as tile
from concourse import mybir
from concourse.bass_utils import run_bass_kernel_spmd

N_CORES = 8
BATCH, SEQ, DIM = 4, 4096, 4096
NUM_SHARDS = 8
ROWS_TOTAL = BATCH * SEQ               # 16384
ROWS_PER_CORE = ROWS_TOTAL // N_CORES  # 2048
P = 128                                # SBUF partitions
T = 4                                  # rows per partition per tile
N_BIG = ROWS_PER_CORE // (T * P)       # 4 tiles

BF16 = ml_dtypes.bfloat16

TRACE = False       # set True (e.g. from test.py) to capture an NTFF profile
LAST_RESULT = None  # BassKernelResults of the most recent kernel() call

_cached_nc = None


def _build_program() -> bass.Bass:
    f32 = mybir.dt.float32
    bf16 = mybir.dt.bfloat16
    nc = bass.Bass()
    x_in = nc.dram_tensor("x", [ROWS_PER_CORE, DIM], bf16, kind="ExternalInput")
    # aux[s, 0:DIM]     = shard_map - s   (bf16-exact: values in [-7, 7])
    # aux[s, DIM:2*DIM] = shards[s, :]
    aux_in = nc.dram_tensor("aux", [NUM_SHARDS, 2 * DIM], bf16,
                            kind="ExternalInput")
    out = nc.dram_tensor("out", [ROWS_PER_CORE, DIM], bf16,
                         kind="ExternalOutput")

    with tile.TileContext(nc) as tc:
        with tc.tile_pool(name="const", bufs=1) as cpool, \
             tc.tile_pool(name="xp", bufs=4) as xpool:
            # aux rides the idle GPSIMD SWDGE ring: the sync HWDGE ring
            # starts streaming x immediately, stores own the ACT ring.
            auxt = cpool.tile([NUM_SHARDS, 2 * DIM], bf16)
            nc.gpsimd.dma_start(auxt[:], aux_in[:])
            ones8 = cpool.tile([NUM_SHARDS, P], bf16)
            nc.vector.memset(ones8[:], 1.0)
            # small SB->SB transfer to warm up the ACT HWDGE ring before
            # the first real store needs it
            warm = cpool.tile([1, 512], bf16)
            nc.vector.memset(warm[:, 0:256], 0.0)
            nc.scalar.dma_start(warm[:, 256:512], warm[:, 0:256])

            # B[s, d] = (shard_map[d] - s == 0) * shards[s, d], in place
            # over the shard_map half of aux.
            nc.vector.scalar_tensor_tensor(
                out=auxt[:, 0:DIM], in0=auxt[:, 0:DIM], scalar=0.0,
                in1=auxt[:, DIM:2 * DIM],
                op0=mybir.AluOpType.is_equal, op1=mybir.AluOpType.mult)

            # w[d] = sum_s B[s, d], replicated to 128 partitions by the
            # ones[8,128] stationary: PSUM[p, d] = sum_s ones[s,p]*B[s,d].
            w128 = cpool.tile([P, DIM], bf16)
            MMF = 512  # one PSUM bank per matmul
            with tc.tile_pool(name="ps", bufs=8, space="PSUM") as ppool:
                for k in range(DIM // MMF):
                    mm = ppool.tile([P, MMF], f32)
                    nc.tensor.matmul(mm[:], ones8[:],
                                     auxt[:, k * MMF:(k + 1) * MMF],
                                     start=True, stop=True)
                    if k % 2 == 0:
                        nc.vector.tensor_copy(w128[:, k * MMF:(k + 1) * MMF],
                                              mm[:])
                    else:
                        nc.scalar.copy(w128[:, k * MMF:(k + 1) * MMF], mm[:])

            # --- stream x through SBUF, scaling by w ---
            # Column chunk r of a tile is row T*p+r, so every [128, 4096]
            # row chunk multiplies against w128 directly.
            x4v = x_in.rearrange("(i p t) d -> i p (t d)", p=P, t=T)
            o4v = out.rearrange("(i p t) d -> i p (t d)", p=P, t=T)
            for i in range(N_BIG):
                xt = xpool.tile([P, T * DIM], bf16)
                first, last = i == 0, i == N_BIG - 1
                nc.sync.dma_start(xt[:], x4v[i])
                for r in range(T):
                    cols = slice(r * DIM, (r + 1) * DIM)
                    nc.vector.tensor_mul(xt[:, cols], xt[:, cols], w128[:])
                    if first or last:
                        # chunked stores: early store start / short drain
                        nc.scalar.dma_start(o4v[i, :, cols], xt[:, cols])
                if not (first or last):
                    nc.scalar.dma_start(o4v[i], xt[:])
    # TRN2 allows one sync wait per instruction; split multi-wait
    # instructions the way bacc's compile pipeline does.
    _bass_rust.generate_event_semaphores(nc)
    return nc


def _marshal(shards: np.ndarray, shard_map: np.ndarray):
    sh = np.asarray(shards, dtype=np.float32)
    sm = np.asarray(shard_map).astype(np.float32)
    aux = np.empty((NUM_SHARDS, 2 * DIM), dtype=BF16)
    aux[:, 0:DIM] = (sm[None, :]
                     - np.arange(NUM_SHARDS, dtype=np.float32)[:, None]
                     ).astype(BF16)
    aux[:, DIM:] = sh.astype(BF16)
    return aux


def kernel(x, shards, shard_map):
    global _cached_nc, LAST_RESULT
    if _cached_nc is None:
        _cached_nc = _build_program()
    nc = _cached_nc

    x2 = np.asarray(x, dtype=np.float32).reshape(ROWS_TOTAL, DIM).astype(BF16)
    aux = _marshal(shards, shard_map)

    in_maps = [
        {"x": x2[c * ROWS_PER_CORE:(c + 1) * ROWS_PER_CORE], "aux": aux}
        for c in range(N_CORES)
    ]
    res = run_bass_kernel_spmd(nc, in_maps, core_ids=list(range(N_CORES)),
                               trace=TRACE)
    LAST_RESULT = res
    out = np.concatenate([r["out"] for r in res.results], axis=0)
    return out.astype(np.float32).reshape(BATCH, SEQ, DIM)


# revision 13
# speedup vs baseline: 1.4577x; 1.0809x over previous
"""Distributed memory-shard scale kernel for Trainium2 (8 NeuronCores).

Computes out[b, s, d] = x[b, s, d] * shards[shard_map[d], d] for
x: [4, 4096, 4096] f32, shards: [8, 4096] f32, shard_map: [4096] int.

Strategy: data-parallel over the flattened (batch*seq) rows — each of the
8 cores owns a contiguous 2048-row slice of x and replicates the tiny
shards/shard_map inputs. The kernel is DMA-bandwidth-bound (pure
elementwise scale), so the x stream is staged in reduced precision and
the host only casts dtypes: dims 0..3327 travel as bf16 and dims
3328..4095 as fp8-e4m3, packed per row into one 7424-byte blob
(measured end-to-end relative error 1.64e-2, inside the 2e-2 budget).
That cuts DMA traffic from 64MB (f32) to 29MB per core.

On device each core:
  1. loads aux[s, :] = [shard_map - s | shards[s]] (bf16, 8 partitions)
     via the otherwise-idle GPSIMD SWDGE ring,
  2. builds masked products B[s, d] = (shard_map[d]==s) * shards[s, d]
     with ONE fused scalar_tensor_tensor, then reduces over shards AND
     broadcasts to all 128 partitions in one step: matmul
     ones[8,128].T @ B[8,512] -> PSUM[128,512] per chunk. PSUM->SBUF bf16
     casts alternate DVE/ACT,
  3. streams x through SBUF in [128, 4*7424] uint8 tiles — four rows per
     partition = 29KB contiguous per-partition lines. (Descriptor size
     matters: SDMA engine 15 processes 16KB descriptors ~20% slower than
     the other 15 engines and paced the whole kernel when tiles had 16KB
     lines; at 8/32KB all 16 engines hit ~26.7 GB/s.) Each row is
     multiplied in two DVE ops through bitcast views: bf16[3328] * w and
     fp8[768] * w. Stores ride the ACT HWDGE ring; the last tile stores
     per row chunk to shorten the final drain.
"""

import numpy as np
import ml_dtypes

import bass_rust as _bass_rust
import concourse.bass as bass
import concourse.tile as tile
from concourse import mybir
from concourse.bass_utils import run_bass_kernel_spmd

N_CORES = 8
BATCH, SEQ, DIM = 4, 4096, 4096
NUM_SHARDS = 8
ROWS_TOTAL = BATCH * SEQ               # 16384
ROWS_PER_CORE = ROWS_TOTAL // N_CORES  # 2048
P = 128                                # SBUF partitions
T = 4                                  # rows per partition per tile
N_BIG = ROWS_PER_CORE // (T * P)       # 4 tiles

C16 = 3328                             # dims carried in bf16
C8 = DIM - C16                         # dims carried in fp8-e4m3 (768)
ROWB = 2 * C16 + C8                    # bytes per packed row (7424)

BF16 = ml_dtypes.bfloat16
FP8 = ml_dtypes.float8_e4m3

TRACE = False       # set True (e.g. from test.py) to capture an NTFF profile
LAST_RESULT = None  # BassKernelResults of the most recent kernel() call

_cached_nc = None


def _build_program() -> bass.Bass:
    f32 = mybir.dt.float32
    bf16 = mybir.dt.bfloat16
    fp8 = mybir.dt.float8e4
    u8 = mybir.dt.uint8
    nc = bass.Bass()
    x_in = nc.dram_tensor("x", [ROWS_PER_CORE, ROWB], u8, kind="ExternalInput")
    # aux[s, 0:DIM]     = shard_map - s   (bf16-exact: values in [-7, 7])
    # aux[s, DIM:2*DIM] = shards[s, :]
    aux_in = nc.dram_tensor("aux", [NUM_SHARDS, 2 * DIM], bf16,
                            kind="ExternalInput")
    out = nc.dram_tensor("out", [ROWS_PER_CORE, ROWB], u8,
                         kind="ExternalOutput")

    with tile.TileContext(nc) as tc:
        with tc.tile_pool(name="const", bufs=1) as cpool, \
             tc.tile_pool(name="xp", bufs=4) as xpool:
            # aux rides the idle GPSIMD SWDGE ring: the sync HWDGE ring
            # starts streaming x immediately, stores own the ACT ring.
            auxt = cpool.tile([NUM_SHARDS, 2 * DIM], bf16)
            nc.gpsimd.dma_start(auxt[:], aux_in[:])
            ones8 = cpool.tile([NUM_SHARDS, P], bf16)
            nc.vector.memset(ones8[:], 1.0)
            # small SB->SB transfer to warm up the ACT HWDGE ring before
            # the first real store needs it
            warm = cpool.tile([1, 512], bf16)
            nc.vector.memset(warm[:, 0:256], 0.0)
            nc.scalar.dma_start(warm[:, 256:512], warm[:, 0:256])

            # B[s, d] = (shard_map[d] - s == 0) * shards[s, d], in place
            # over the shard_map half of aux.
            nc.vector.scalar_tensor_tensor(
                out=auxt[:, 0:DIM], in0=auxt[:, 0:DIM], scalar=0.0,
                in1=auxt[:, DIM:2 * DIM],
                op0=mybir.AluOpType.is_equal, op1=mybir.AluOpType.mult)

            # w[d] = sum_s B[s, d], replicated to 128 partitions by the
            # ones[8,128] stationary: PSUM[p, d] = sum_s ones[s,p]*B[s,d].
            w128 = cpool.tile([P, DIM], bf16)
            MMF = 512  # one PSUM bank per matmul
            with tc.tile_pool(name="ps", bufs=8, space="PSUM") as ppool:
                for k in range(DIM // MMF):
                    mm = ppool.tile([P, MMF], f32)
                    nc.tensor.matmul(mm[:], ones8[:],
                                     auxt[:, k * MMF:(k + 1) * MMF],
                                     start=True, stop=True)
                    if k % 2 == 0:
                        nc.vector.tensor_copy(w128[:, k * MMF:(k + 1) * MMF],
                                              mm[:])
                    else:
                        nc.scalar.copy(w128[:, k * MMF:(k + 1) * MMF], mm[:])

            # --- stream x through SBUF, scaling by w ---
            # Byte-column chunk r of a tile is packed row T*p+r.
            x4v = x_in.rearrange("(i p t) b -> i p (t b)", p=P, t=T)
            o4v = out.rearrange("(i p t) b -> i p (t b)", p=P, t=T)
            for i in range(N_BIG):
                xt = xpool.tile([P, T * ROWB], u8)
                last = i == N_BIG - 1
                nc.sync.dma_start(xt[:], x4v[i])
                for r in range(T):
                    v16 = xt[:, r * ROWB:r * ROWB + 2 * C16].bitcast(bf16)
                    v8 = xt[:, r * ROWB + 2 * C16:(r + 1) * ROWB].bitcast(fp8)
                    nc.vector.tensor_mul(v16, v16, w128[:, 0:C16])
                    nc.vector.tensor_mul(v8, v8, w128[:, C16:DIM])
                    if last:
                        cols = slice(r * ROWB, (r + 1) * ROWB)
                        nc.scalar.dma_start(o4v[i, :, cols], xt[:, cols])
                if not last:
                    nc.scalar.dma_start(o4v[i], xt[:])
    # TRN2 allows one sync wait per instruction; split multi-wait
    # instructions the way bacc's compile pipeline does.
    _bass_rust.generate_event_semaphores(nc)
    return nc


def _marshal(shards: np.ndarray, shard_map: np.ndarray):
    sh = np.asarray(shards, dtype=np.float32)
    sm = np.asarray(shard_map).astype(np.float32)
    aux = np.empty((NUM_SHARDS, 2 * DIM), dtype=BF16)
    aux[:, 0:DIM] = (sm[None, :]
                     - np.arange(NUM_SHARDS, dtype=np.float32)[:, None]
                     ).astype(BF16)
    aux[:, DIM:] = sh.astype(BF16)
    return aux


def _pack_x(x: np.ndarray) -> np.ndarray:
    x2 = np.asarray(x, dtype=np.float32).reshape(ROWS_TOTAL, DIM)
    xb = np.empty((ROWS_TOTAL, ROWB), dtype=np.uint8)
    xb[:, :2 * C16] = x2[:, :C16].astype(BF16).view(np.uint8)
    xb[:, 2 * C16:] = x2[:, C16:].astype(FP8).view(np.uint8)
    return xb


def _unpack_out(ob: np.ndarray) -> np.ndarray:
    o = np.empty((ROWS_TOTAL, DIM), dtype=np.float32)
    o[:, :C16] = np.ascontiguousarray(ob[:, :2 * C16]).view(BF16)
    o[:, C16:] = np.ascontiguousarray(ob[:, 2 * C16:]).view(FP8)
    return o


def kernel(x, shards, shard_map):
    global _cached_nc, LAST_RESULT
    if _cached_nc is None:
        _cached_nc = _build_program()
    nc = _cached_nc

    xb = _pack_x(x)
    aux = _marshal(shards, shard_map)

    in_maps = [
        {"x": xb[c * ROWS_PER_CORE:(c + 1) * ROWS_PER_CORE], "aux": aux}
        for c in range(N_CORES)
    ]
    res = run_bass_kernel_spmd(nc, in_maps, core_ids=list(range(N_CORES)),
                               trace=TRACE)
    LAST_RESULT = res
    ob = np.concatenate([r["out"] for r in res.results], axis=0)
    return _unpack_out(ob).reshape(BATCH, SEQ, DIM)
